# revision 4
# baseline (speedup 1.0000x reference)
"""NCA Perception 2D kernel for 8x Trainium2 NeuronCores.

Data-parallel over batch: image i runs on core i. Each core computes, for its
(64, 256, 256) image:
  - sep  (192 ch): depthwise separable 3x3 filters (identity, sobel_y, sobel_x)
  - lap  (64 ch):  depthwise 4-connected Laplacian
  - learned (128 ch): dense 3x3 conv (64->128) + bias, via f32r matmuls on PE
with replicate (edge) padding, concatenated to (384, 256, 256).

SBUF layout per 16-row block: x tile [128, 2562] where col 0 / col 2561 are
zero pads and cols 1..2560 hold 10 row-slots of 256 px for both partition
halves (partitions 0-63: channels for rows y0-1..y0+8; 64-127: rows
y0+7..y0+16). Rows are stored unpadded/contiguous; the horizontal conv taps
read 1-shifted views, which wrap across row boundaries, and the wrapped first/
last column of every row is then corrected (exactly) with per-column fixes:
depthwise via tiny DVE ops, learned via 12 extra N=8 matmuls per half using
+/- edge-tap weights.
"""

import numpy as np

import concourse.bacc as bacc
import concourse.bass as bass
import concourse.mybir as mybir
from concourse.tile import TileContext
from concourse.bass_utils import run_bass_kernel_spmd

F32 = mybir.dt.float32
F32R = mybir.dt.float32r
ADD = mybir.AluOpType.add
SUB = mybir.AluOpType.subtract
MULT = mybir.AluOpType.mult

C, H, W, F = 64, 256, 256, 128
R = 16              # image rows per block
HB = R // 2         # rows per partition-half
S = HB + 2          # row slots per half (incl. 1-row halo each side)
NBLK = H // R
HWsz = H * W
XCOLS = S * W + 2   # front pad + 10*256 + back pad

# wb column layout: [2 variants x 9 taps x 128] | [2 variants x 6 neg edge taps x 128]
#                   | bias 128 | ones 256
NEG_OFF = 2 * 9 * F
BIAS_OFF = NEG_OFF + 2 * 6 * F
ONES_OFF = BIAS_OFF + F
WB_COLS = ONES_OFF + W
# neg block tap order per variant: dy0,dy1,dy2 of dx=0 then dy0,dy1,dy2 of dx=2


def build_nc(n_iter=1):
    nc = bacc.Bacc(name="nca")
    x = nc.dram_tensor("x", [C, H, W], F32R, kind="ExternalInput")
    wb = nc.dram_tensor("wb", [128, WB_COLS], F32R, kind="ExternalInput")
    out = nc.dram_tensor("out", [384, H, W], F32, kind="ExternalOutput")

    with TileContext(nc) as tc:
        with (
            tc.tile_pool(name="wpool", bufs=1) as wpool,
            tc.tile_pool(name="xpool", bufs=2) as xpool,
            tc.tile_pool(name="ipool", bufs=1) as ipool,
            tc.tile_pool(name="opool", bufs=2) as opool,
            tc.tile_pool(name="psum", bufs=4, space="PSUM") as psum_pool,
            tc.tile_pool(name="psumc", bufs=2, space="PSUM") as psumc_pool,
        ):
            wbt = wpool.tile([128, WB_COLS], F32R)
            nc.sync.dma_start(out=wbt[:], in_=wb[:])

            def block_body(blk):
                y0 = blk * R
                xt = xpool.tile([128, XCOLS], F32R)
                xtf = xt[:].bitcast(F32)

                # ---- input loads ----
                nc.gpsimd.memset(xt[:, 0:1].bitcast(F32), 0.0)
                nc.gpsimd.memset(xt[:, XCOLS - 1:XCOLS].bitcast(F32), 0.0)
                if 0 < blk < NBLK - 1:
                    nc.sync.dma_start(
                        out=xt[:, 1:1 + S * W],
                        in_=bass.AP(x, (y0 - 1) * W, [[HB * W, 2], [HWsz, C], [1, S * W]]),
                    )
                elif blk == 0:
                    nc.sync.dma_start(
                        out=xt[0:C, 1 + W:1 + S * W],
                        in_=bass.AP(x, 0, [[HWsz, C], [1, (S - 1) * W]]),
                    )
                    nc.sync.dma_start(out=xt[0:C, 1:1 + W], in_=xt[0:C, 1 + W:1 + 2 * W])
                    nc.sync.dma_start(
                        out=xt[C:128, 1:1 + S * W],
                        in_=bass.AP(x, (HB - 1) * W, [[HWsz, C], [1, S * W]]),
                    )
                else:
                    nc.sync.dma_start(
                        out=xt[0:C, 1:1 + S * W],
                        in_=bass.AP(x, (y0 - 1) * W, [[HWsz, C], [1, S * W]]),
                    )
                    nc.sync.dma_start(
                        out=xt[C:128, 1:1 + (S - 1) * W],
                        in_=bass.AP(x, (y0 + HB - 1) * W, [[HWsz, C], [1, (S - 1) * W]]),
                    )
                    nc.sync.dma_start(
                        out=xt[C:128, 1 + (S - 1) * W:1 + S * W],
                        in_=xt[C:128, 1 + (S - 2) * W:1 + (S - 1) * W],
                    )

                # ---- learned conv: 10 f32r matmuls per output row ----
                lt = opool.tile([128, R * W], F32)
                for half in (0, 1):
                    woff = half * 9 * F
                    for i in range(HB):
                        ps = psum_pool.tile([128, W], F32)
                        nc.tensor.matmul(
                            ps[:],
                            lhsT=wbt[0:1, BIAS_OFF:BIAS_OFF + F],
                            rhs=wbt[0:1, ONES_OFF:ONES_OFF + W],
                            start=True, stop=False,
                        )
                        for dy in range(3):
                            for dx in range(3):
                                col = (i + dy) * W + dx
                                t = dy * 3 + dx
                                nc.tensor.matmul(
                                    ps[:],
                                    lhsT=wbt[:, woff + t * F:woff + (t + 1) * F],
                                    rhs=xt[:, col:col + W],
                                    start=False, stop=(t == 8),
                                )
                        pos = half * HB + i
                        nc.scalar.copy(out=lt[:, pos * W:(pos + 1) * W], in_=ps[:])

                    # edge-column corrections: psc0 fixes col 0, psc1 fixes col 255
                    noff = NEG_OFF + half * 6 * F
                    psc0 = psumc_pool.tile([128, HB], F32)
                    psc1 = psumc_pool.tile([128, HB], F32)
                    for dy in range(3):
                        t = dy * 3 + 0
                        a = 1 + dy * W
                        nc.tensor.matmul(
                            psc0[:],
                            lhsT=wbt[:, woff + t * F:woff + (t + 1) * F],
                            rhs=xt[:, a:a + 7 * W + 1:W],
                            start=(dy == 0), stop=False,
                        )
                    for dy in range(3):
                        a = dy * W
                        nc.tensor.matmul(
                            psc0[:],
                            lhsT=wbt[:, noff + dy * F:noff + (dy + 1) * F],
                            rhs=xt[:, a:a + 7 * W + 1:W],
                            start=False, stop=(dy == 2),
                        )
                    for dy in range(3):
                        t = dy * 3 + 2
                        a = (dy + 1) * W
                        nc.tensor.matmul(
                            psc1[:],
                            lhsT=wbt[:, woff + t * F:woff + (t + 1) * F],
                            rhs=xt[:, a:a + 7 * W + 1:W],
                            start=(dy == 0), stop=False,
                        )
                    for dy in range(3):
                        a = 1 + (dy + 1) * W
                        nc.tensor.matmul(
                            psc1[:],
                            lhsT=wbt[:, noff + (3 + dy) * F:noff + (4 + dy) * F],
                            rhs=xt[:, a:a + 7 * W + 1:W],
                            start=False, stop=(dy == 2),
                        )
                    lt3 = lt[:].rearrange("p (r c) -> p r c", c=W)
                    rows = slice(half * HB, half * HB + HB)
                    nc.vector.tensor_add(
                        lt3[:, rows, 0:1], lt3[:, rows, 0:1], psc0[:].unsqueeze(2))
                    nc.vector.tensor_add(
                        lt3[:, rows, W - 1:W], lt3[:, rows, W - 1:W], psc1[:].unsqueeze(2))

                nc.sync.dma_start(out=out[256:384, y0:y0 + R, :], in_=lt[:])

                # ---- depthwise on DVE ----
                dd = ipool.tile([128, S], F32)
                ee = ipool.tile([128, S], F32)
                nc.vector.tensor_sub(dd[:], xtf[:, 1:1 + (S - 1) * W + 1:W],
                                     xtf[:, 0:(S - 1) * W + 1:W])
                nc.vector.tensor_sub(ee[:], xtf[:, W:S * W + 1:W],
                                     xtf[:, W + 1:S * W + 2:W])
                d3 = dd[:].unsqueeze(2)
                e3 = ee[:].unsqueeze(2)

                tt = ipool.tile([128, S * W], F32)
                v1 = ipool.tile([128, S * W], F32)
                v2 = ipool.tile([128, S * W], F32)
                ww = ipool.tile([128, (S - 1) * W], F32)
                t3 = tt[:].rearrange("p (s c) -> p s c", c=W)
                v23 = v2[:].rearrange("p (s c) -> p s c", c=W)

                nc.vector.tensor_add(tt[:], xtf[:, 0:S * W], xtf[:, 2:S * W + 2])
                nc.vector.tensor_add(t3[:, :, 0:1], t3[:, :, 0:1], d3)
                nc.vector.tensor_add(t3[:, :, W - 1:W], t3[:, :, W - 1:W], e3)
                nc.vector.scalar_tensor_tensor(
                    v1[:], xtf[:, 1:S * W + 1], 2.0, tt[:], MULT, ADD)
                nc.vector.tensor_sub(v2[:], xtf[:, 0:S * W], xtf[:, 2:S * W + 2])
                nc.vector.tensor_add(v23[:, :, 0:1], v23[:, :, 0:1], d3)
                nc.vector.tensor_sub(v23[:, :, W - 1:W], v23[:, :, W - 1:W], e3)
                nc.vector.tensor_add(ww[:], v2[:, 0:(S - 1) * W], v2[:, W:S * W])

                sp1 = opool.tile([128, HB * W], F32)
                sp2 = opool.tile([128, HB * W], F32)
                lp = opool.tile([128, HB * W], F32)
                nc.vector.tensor_sub(sp1[:], v1[:, 0:HB * W], v1[:, 2 * W:S * W])
                nc.vector.tensor_add(sp2[:], ww[:, 0:HB * W], ww[:, W:(S - 1) * W])

                ud = ipool.tile([128, HB * W], F32)
                sm = ipool.tile([128, HB * W], F32)
                nc.vector.tensor_add(ud[:], xtf[:, 1:1 + HB * W], xtf[:, 1 + 2 * W:1 + S * W])
                nc.vector.tensor_add(sm[:], ud[:], tt[:, W:(S - 1) * W])
                nc.vector.scalar_tensor_tensor(
                    lp[:], xtf[:, 1 + W:1 + (S - 1) * W], 4.0, sm[:], MULT, SUB)

                # ---- stores: sep0 (copy of x), sep1, sep2, lap ----
                sep_dims = [[HB * W, 2], [3 * HWsz, C], [1, HB * W]]
                nc.sync.dma_start(
                    out=bass.AP(out, 0 * HWsz + y0 * W, sep_dims),
                    in_=xtf[:, 1 + W:1 + (S - 1) * W],
                )
                nc.sync.dma_start(
                    out=bass.AP(out, 1 * HWsz + y0 * W, sep_dims), in_=sp1[:])
                nc.sync.dma_start(
                    out=bass.AP(out, 2 * HWsz + y0 * W, sep_dims), in_=sp2[:])
                nc.sync.dma_start(
                    out=bass.AP(out, 192 * HWsz + y0 * W, [[HB * W, 2], [HWsz, C], [1, HB * W]]),
                    in_=lp[:])

            if n_iter == 1:
                for blk in range(NBLK):
                    block_body(blk)
            else:
                def body(_iv):
                    for blk in range(NBLK):
                        block_body(blk)
                tc.For_i_unrolled(0, n_iter, 1, body, max_unroll=1)

    nc.finalize()
    return nc


def pack_wb(W_learned, b_learned):
    wb = np.zeros((128, WB_COLS), dtype=np.float32)
    # W_learned: (F, C, 3, 3) -> lhsT[k=c, m=f] = W[f, c, dy, dx]
    wt = np.transpose(W_learned, (2, 3, 1, 0)).reshape(9, C, F)  # (tap, c, f)
    for half in (0, 1):
        rows = slice(half * C, half * C + C)
        for t in range(9):
            wb[rows, half * 9 * F + t * F:half * 9 * F + (t + 1) * F] = wt[t]
        noff = NEG_OFF + half * 6 * F
        for dy in range(3):
            wb[rows, noff + dy * F:noff + (dy + 1) * F] = -wt[dy * 3 + 0]
            wb[rows, noff + (3 + dy) * F:noff + (4 + dy) * F] = -wt[dy * 3 + 2]
    wb[0, BIAS_OFF:BIAS_OFF + F] = b_learned
    wb[0, ONES_OFF:ONES_OFF + W] = 1.0
    return wb


_NC_CACHE = {}


def kernel(x, W_learned, b_learned):
    x = np.ascontiguousarray(np.asarray(x, dtype=np.float32))
    W_learned = np.asarray(W_learned, dtype=np.float32)
    b_learned = np.asarray(b_learned, dtype=np.float32)
    N = x.shape[0]
    assert N == 8 and x.shape[1:] == (C, H, W)

    if "nc" not in _NC_CACHE:
        _NC_CACHE["nc"] = build_nc()
    nc = _NC_CACHE["nc"]

    wb = pack_wb(W_learned, b_learned)
    in_maps = [{"x": x[i], "wb": wb} for i in range(N)]
    res = run_bass_kernel_spmd(nc, in_maps, list(range(N)))
    return np.stack([res.results[i]["out"] for i in range(N)], axis=0)
